# revision 23
# baseline (speedup 1.0000x reference)
# Trainium2 Bass kernel for nn_Seq2Seq (GRU encoder-decoder, batch=1).
#
# Architecture choices:
# - Single NeuronCore does everything: the 4096-step sequential recurrence
#   dominates (measured ~2.5us/step); per-step cross-core collectives would
#   be far slower than the step itself, so tensor-parallelism loses here.
# - All matmul operands in fp16 (PE runs fp32 matmuls at 1/4 rate; fp16 also
#   enables fast-weight-load and 16-bit DMA-transpose; fp16 mantissa keeps
#   end-to-end rel err ~5e-4). Accumulation is fp32 in PSUM.
# - Scan layout is "hidden-on-partitions": a 256-vector is an SBUF tile
#   [128, 2]. Per step, DVE preloads x-projection (+biases) into PSUM
#   (walrus's dummy-matmul has_written workaround makes the subsequent
#   accumulating matmuls correct), then 12 [128,128] weight-stationary
#   matmuls accumulate W_hh @ h on top.
# - r/z/n gate parts use three separate PSUM banks so bank-level dependency
#   tracking lets sigmoid(r) start while n/z matmuls still run.
# - The inner 16-step block uses only static SBUF offsets; each For_i
#   iteration stages xp/xn slices in (and the hidden-state block out) with
#   one dynamic copy per tensor, respecting the program-wide ~12 dynamic-AP
#   register budget per engine.
# - _split_multi_waits post-pass: this container's walrus accepts at most
#   one semaphore wait per instruction, so extra waits are moved onto
#   preceding NoOp carrier instructions.
import os
import numpy as np

T = 2048
I = 1739
IP = 1792  # I padded to 14*128
KI = IP // 128  # 14
H = 256
G3 = 768

_CACHE = {}



def _split_multi_waits(nc, mybir):
    """walrus codegen in this container accepts at most one sem-wait per
    instruction; move extra waits onto preceding NoOp carriers."""
    k = 0
    for fn in nc.m.functions:
        for bb in fn.blocks:
            insts = bb.instructions
            out = []
            changed = False
            for inst in insts:
                si = inst.sync_info
                if si is not None and si.on_wait and len(si.on_wait) > 1:
                    waits = list(si.on_wait)
                    for w in waits[:-1]:
                        nop = mybir.InstNoOp(name=f"nopw-{k}", ins=[], outs=[])
                        k += 1
                        nop.engine = inst.engine
                        nop.sync_info = mybir.SyncInfo(on_wait=[w], on_update=[])
                        out.append(nop)
                    inst.sync_info = mybir.SyncInfo(
                        on_wait=[waits[-1]], on_update=list(si.on_update or [])
                    )
                    changed = True
                out.append(inst)
            if changed:
                insts[:] = out


def _build(nc, bass, mybir, tile, T, U):
    from contextlib import ExitStack

    f32 = mybir.dt.float32
    f16 = mybir.dt.float16
    AF = mybir.ActivationFunctionType
    Alu = mybir.AluOpType

    x_in = nc.dram_tensor("input_trajectory", [1, T, I], f32, kind="ExternalInput")
    wih = {}
    whh = {}
    bih = {}
    bhh = {}
    for g in ("enc", "dec"):
        wih[g] = nc.dram_tensor(f"W_ih_{g}", [G3, I], f32, kind="ExternalInput")
        whh[g] = nc.dram_tensor(f"W_hh_{g}", [G3, H], f32, kind="ExternalInput")
        bih[g] = nc.dram_tensor(f"b_ih_{g}", [G3], f32, kind="ExternalInput")
        bhh[g] = nc.dram_tensor(f"b_hh_{g}", [G3], f32, kind="ExternalInput")
    wout = nc.dram_tensor("W_out", [I, H], f32, kind="ExternalInput")
    bout = nc.dram_tensor("b_out", [I], f32, kind="ExternalInput")
    out = nc.dram_tensor("output", [T, 1, I], f32, kind="ExternalOutput")

    ident = nc.inline_tensor(np.eye(128, dtype=np.float16), name="ident128")
    ones_row = nc.inline_tensor(np.ones((1, 128), dtype=np.float16), name="ones_row")

    NTB = T // 512 if T >= 512 else 1  # t-blocks for x-projection
    TBW = min(T, 512)

    with tile.TileContext(nc) as tc, ExitStack() as top:
        dram = top.enter_context(tc.tile_pool(name="dram", bufs=1, space="DRAM"))
        const = top.enter_context(tc.tile_pool(name="const", bufs=1))

        x16 = dram.tile([T, IP], f16)
        wih16 = {g: dram.tile([G3, IP], f16, tag=f"wih16_{g}", name=f"wih16_{g}") for g in ("enc", "dec")}
        whh16 = {g: dram.tile([G3, H], f16, tag=f"whh16_{g}", name=f"whh16_{g}") for g in ("enc", "dec")}
        wout16 = dram.tile([IP, H], f16)

        # ---- constants: identity, ones, biases ----
        ident_sb = const.tile([128, 128], f16, tag="ident")
        nc.sync.dma_start(ident_sb[:], ident[:])
        ones_sb = const.tile([1, 128], f16, tag="ones")
        nc.sync.dma_start(ones_sb[:], ones_row[:])

        # per-partition bias tiles [128, 6]: col m holds b[m*128 + p]
        bias_rz = {}  # b_ih + b_hh (used for r,z cols 0:4)
        bias_n = {}  # b_ih only (col m=4,5 for xn)
        bhhn16 = {}  # b_hh n-part as f16 [128,2]
        for g in ("enc", "dec"):
            bi = const.tile([128, 6], f32, tag=f"bi_{g}")
            bh = const.tile([128, 6], f32, tag=f"bh_{g}")
            nc.gpsimd.dma_start(bi[:], bih[g][:].rearrange("(m p) -> p m", p=128))
            nc.gpsimd.dma_start(bh[:], bhh[g][:].rearrange("(m p) -> p m", p=128))
            bc = const.tile([128, 6], f32, tag=f"bc_{g}")
            nc.vector.tensor_add(bc[:], bi[:], bh[:])
            bn16 = const.tile([128, 2], f16, tag=f"bn16_{g}")
            nc.vector.tensor_copy(bn16[:], bh[:, 4:6])
            bias_rz[g] = bc
            bias_n[g] = bi
            bhhn16[g] = bn16

        # b_out as f16 row [1, I]
        bout16 = const.tile([1, I], f16, tag="bout16")
        with tc.tile_pool(name="bo32", bufs=1) as bo32p:
            bo32 = bo32p.tile([1, I], f32)
            nc.gpsimd.dma_start(bo32[:], bout[:].rearrange("(o i) -> o i", o=1))
            nc.vector.tensor_copy(bout16[:], bo32[:])

        # ---- fp32 -> fp16 conversion into padded DRAM scratch ----
        def convert(src2d, dst, rows, cols, cols_pad):
            with (
                tc.tile_pool(name="cv32", bufs=3) as p32,
                tc.tile_pool(name="cv16", bufs=3) as p16,
            ):
                for r0 in range(0, rows, 128):
                    rr = min(128, rows - r0)
                    t32 = p32.tile([128, cols], f32)
                    t16 = p16.tile([128, cols_pad], f16)
                    nc.sync.dma_start(t32[:rr, :], src2d[r0 : r0 + rr, :])
                    if rr < 128:
                        nc.vector.memset(t16[:], 0.0)
                    elif cols_pad > cols:
                        nc.vector.memset(t16[:, cols:], 0.0)
                    nc.vector.tensor_copy(t16[:rr, :cols], t32[:rr, :])
                    re = min(r0 + 128, dst.shape[0])
                    nc.sync.dma_start(dst[r0:re, :], t16[: re - r0, :])

        convert(x_in[:].rearrange("o t i -> (o t) i"), x16[:], T, I, IP)
        for g in ("enc", "dec"):
            convert(wih[g][:], wih16[g][:], G3, I, IP)
            convert(whh[g][:], whh16[g][:], G3, H, H)
        convert(wout[:], wout16[:], I, H, H)

        # warm the ACT table set (sigmoid & tanh live in the same set)
        with tc.tile_pool(name="warm", bufs=1) as wp:
            wt = wp.tile([128, 2], f32)
            nc.vector.memset(wt[:], 0.0)
            nc.scalar.activation(wt[:], wt[:], AF.Sigmoid)
            nc.scalar.activation(wt[:], wt[:], AF.Tanh)

        # persistent hs buffer (dec hidden states h_1..h_T), fp16
        hs = const.tile([128, T * 2], f16, tag="hs")
        hs_v = hs.rearrange("p (t c) -> p t c", c=2)
        carry = const.tile([128, 2], f16, tag="carry")

        def xproj(g, xp_pool, xn_pool):
            """Builds xp (f16, [128, T*6] interleaved) and xn (f32, [128, T*2])."""
            xp = xp_pool.tile([128, T * 6], f16, tag=f"xp_{g}")
            xn = xn_pool.tile([128, T * 2], f32, tag=f"xn_{g}")
            xp_v = xp[:].rearrange("p (t m) -> p t m", m=6)
            xn_v = xn[:].rearrange("p (t c) -> p t c", c=2)

            with (
                tc.tile_pool(name="wihT", bufs=1) as wtp,
                tc.tile_pool(name="xT", bufs=3) as xtp,
                tc.tile_pool(name="psA", bufs=1, space="PSUM") as psA,
            ):
                wT = []
                for j in range(KI):
                    w = wtp.tile([128, G3], f16, tag=f"wT{j}", name=f"wT{j}")
                    nc.sync.dma_start_transpose(
                        out=w[:], in_=wih16[g][:, j * 128 : (j + 1) * 128]
                    )
                    wT.append(w)
                for mp in range(3):
                    pts = [
                        [psA.tile([128, TBW], f32, tag=f"pt{mi}_{tb}", name=f"pt{mi}_{tb}") for tb in range(NTB)]
                        for mi in range(2)
                    ]
                    for j in range(KI):
                        xT = xtp.tile([128, T], f16, tag="xT")
                        nc.sync.dma_start_transpose(
                            out=xT[:], in_=x16[:, j * 128 : (j + 1) * 128]
                        )
                        for mi in range(2):
                            m = 2 * mp + mi
                            for tb in range(NTB):
                                nc.tensor.matmul(
                                    pts[mi][tb][:],
                                    wT[j][:, m * 128 : (m + 1) * 128],
                                    xT[:, tb * TBW : (tb + 1) * TBW],
                                    start=(j == 0),
                                    stop=(j == KI - 1),
                                )
                    for mi in range(2):
                        m = 2 * mp + mi
                        for tb in range(NTB):
                            tsl = slice(tb * TBW, (tb + 1) * TBW)
                            if m < 4:
                                nc.scalar.activation(
                                    xp_v[:, tsl, m],
                                    pts[mi][tb][:],
                                    AF.Identity,
                                    bias=bias_rz[g][:, m : m + 1],
                                )
                            else:
                                nc.scalar.activation(
                                    xn_v[:, tsl, m - 4],
                                    pts[mi][tb][:],
                                    AF.Identity,
                                    bias=bias_n[g][:, m : m + 1],
                                )
            # fill xp cols 4:6 with b_hh_n via log-doubling
            nc.vector.tensor_copy(xp_v[:, 0, 4:6], bhhn16[g][:])
            k = 1
            while k < T:
                kk = min(k, T - k)
                nc.vector.tensor_copy(xp_v[:, k : k + kk, 4:6], xp_v[:, 0:kk, 4:6])
                k *= 2
            return xp[:], xn[:]

        def scan(g, xp_f, xn_f, is_dec, carry):
            whhT = const.tile([128, 2 * G3], f16, tag=f"whhT_{g}", name=f"whhT_{g}")
            for c in range(2):
                nc.sync.dma_start_transpose(
                    out=whhT[:, c * G3 : (c + 1) * G3],
                    in_=whh16[g][:, c * 128 : (c + 1) * 128],
                )

            with (
                tc.tile_pool(name="stg", bufs=2) as stg,
                tc.tile_pool(name="hbp", bufs=2) as hbp,
                tc.tile_pool(name="psS", bufs=2, space="PSUM") as psS,
                tc.tile_pool(name="gt", bufs=4) as gt,
            ):

                _dvepre = bool(int(os.environ.get("DVEPRE", "1")))
                _rzjoint = bool(int(os.environ.get("RZJOINT", "0")))
                _tanhfuse = bool(int(os.environ.get("TANHFUSE", "0")))

                def preload(ps, u, m0, width, xps):
                    if _dvepre:
                        nc.vector.tensor_copy(ps[:], xps[:, u * 6 + m0 : u * 6 + m0 + width])
                    else:
                        nc.tensor.matmul(
                            ps[:], ident_sb[:], xps[:, u * 6 + m0 : u * 6 + m0 + width],
                            start=True, stop=False,
                        )

                def wmms(ps, col, m, h_prev, last):
                    for c in range(2):
                        nc.tensor.matmul(
                            ps[:, col : col + 1],
                            whhT[:, c * G3 + m * 128 : c * G3 + (m + 1) * 128],
                            h_prev[:, c : c + 1],
                            start=False,
                            stop=(last and c == 1),
                        )

                def step(u, xps, xns, h_prev, h_new):
                    w = gt.tile([128, 2], f32, tag="w", name="w")
                    v = gt.tile([128, 2], f32, tag="v", name="v")
                    n = gt.tile([128, 2], f32, tag="n", name="n")
                    d = gt.tile([128, 2], f32, tag="d", name="d")
                    e = gt.tile([128, 2], f32, tag="e", name="e")
                    ps_n = psS.tile([128, 2], f32, tag="ps_n", name="ps_n")
                    if _rzjoint:
                        # r and z share one PSUM bank; one sigmoid over [128,4]
                        ps_rz = psS.tile([128, 4], f32, tag="ps_rz", name="ps_rz")
                        rz = gt.tile([128, 4], f32, tag="rz", name="rz")
                        preload(ps_rz, u, 0, 4, xps)
                        for mi in range(4):
                            wmms(ps_rz, mi, mi, h_prev, mi == 3)
                        preload(ps_n, u, 4, 2, xps)
                        for mi in range(2):
                            wmms(ps_n, mi, 4 + mi, h_prev, mi == 1)
                        nc.scalar.activation(rz[:], ps_rz[:], AF.Sigmoid)
                        r, z = rz[:, 0:2], rz[:, 2:4]
                        nc.vector.tensor_mul(w[:], r, ps_n[:])
                        nc.vector.tensor_add(v[:], w[:], xns[:, u * 2 : u * 2 + 2])
                        nc.scalar.activation(n[:], v[:], AF.Tanh)
                        nc.vector.tensor_sub(d[:], h_prev[:], n[:])
                        nc.vector.tensor_mul(e[:], z, d[:])
                        nc.vector.tensor_add(h_new[:], n[:], e[:])
                        return
                    r = gt.tile([128, 2], f32, tag="r", name="r")
                    z = gt.tile([128, 2], f32, tag="z", name="z")
                    ps_r = psS.tile([128, 2], f32, tag="ps_r", name="ps_r")
                    ps_z = psS.tile([128, 2], f32, tag="ps_z", name="ps_z")
                    _zearly = bool(int(os.environ.get("ZEARLY", "0")))
                    if _zearly:
                        parts = (("r", ps_r, 0), ("z", ps_z, 2), ("n", ps_n, 4))
                    else:
                        parts = (("r", ps_r, 0), ("n", ps_n, 4), ("z", ps_z, 2))
                    for part, ps, m0 in parts:
                        preload(ps, u, m0, 2, xps)
                        for mi in range(2):
                            wmms(ps, mi, m0 + mi, h_prev, mi == 1)
                    nc.scalar.activation(r[:], ps_r[:], AF.Sigmoid)
                    if _zearly:
                        nc.scalar.activation(z[:], ps_z[:], AF.Sigmoid)
                    if _tanhfuse:
                        # n = tanh(r * (W_hh@h + b_hh_n) + xn), fused per half
                        # (ACT scale/bias operands are per-partition [128,1])
                        for c in range(2):
                            nc.scalar.activation(
                                n[:, c : c + 1],
                                ps_n[:, c : c + 1],
                                AF.Tanh,
                                bias=xns[:, u * 2 + c : u * 2 + c + 1],
                                scale=r[:, c : c + 1],
                            )
                    else:
                        nc.vector.tensor_mul(w[:], r[:], ps_n[:])
                        nc.vector.tensor_add(v[:], w[:], xns[:, u * 2 : u * 2 + 2])
                        nc.scalar.activation(n[:], v[:], AF.Tanh)
                    if not _zearly:
                        nc.scalar.activation(z[:], ps_z[:], AF.Sigmoid)
                    nc.vector.tensor_sub(d[:], h_prev[:], n[:])
                    nc.vector.tensor_mul(e[:], z[:], d[:])
                    nc.vector.tensor_add(h_new[:], n[:], e[:])

                _mult = int(os.environ.get("SCAN_MULT", "1"))
                _nb = T // U
                _sr = bool(int(os.environ.get("STAG", "1")))
                _gps = bool(int(os.environ.get("GPSTAGE", "1")))
                seng = nc.gpsimd if _gps else nc.vector
                with tc.For_i(0, _nb * _mult, staggered_reset=_sr) as ib0:
                    ib = ib0 if _mult == 1 else tile.smin(ib0, _nb - 1)
                    xps = stg.tile([128, U * 6], f16, tag="xps", name="xps")
                    xns = stg.tile([128, U * 2], f32, tag="xns", name="xns")
                    seng.tensor_copy(xps[:], xp_f[:, bass.ds(ib * (U * 6), U * 6)])
                    seng.tensor_copy(xns[:], xn_f[:, bass.ds(ib * (U * 2), U * 2)])
                    hb = hbp.tile([128, U * 2], f16, tag="hb", name="hb")
                    for u in range(U):
                        h_prev = carry if u == 0 else hb[:, (u - 1) * 2 : u * 2]
                        h_new = hb[:, u * 2 : (u + 1) * 2]
                        step(u, xps, xns, h_prev, h_new)
                    if is_dec:
                        seng.tensor_copy(
                            hs[:, bass.ds(ib * (U * 2), U * 2)], hb[:]
                        )
                    nc.vector.tensor_copy(carry[:], hb[:, (U - 1) * 2 :])

        # ---------- encoder + decoder ----------
        skip_scan = bool(int(os.environ.get("SKIP_SCAN", "0")))
        n_repeat = int(os.environ.get("SCAN_REPEAT", "1"))
        if skip_scan:
            nc.vector.memset(hs[:], 0.0)
        with (
            tc.tile_pool(name="xpE", bufs=1) as xpE,
            tc.tile_pool(name="xnE", bufs=1) as xnE,
            tc.tile_pool(name="xpD", bufs=1) as xpD,
            tc.tile_pool(name="xnD", bufs=1) as xnD,
        ):
            xpe_f, xne_f = xproj("enc", xpE, xnE)
            xpd_f, xnd_f = xproj("dec", xpD, xnD)
            if not skip_scan:
                for _rep in range(n_repeat):
                    nc.vector.memset(carry[:], 0.0)
                    scan("enc", xpe_f, xne_f, False, carry)
                    scan("dec", xpd_f, xnd_f, True, carry)

        # ---------- output projection: preds = hs @ W_out.T + b_out ----------
        out_v = out[:].rearrange("t o i -> (t o) i")
        with (
            tc.tile_pool(name="woT", bufs=1) as woT,
            tc.tile_pool(name="psO", bufs=4, space="PSUM") as psO,
            tc.tile_pool(name="ot", bufs=4) as ot,
        ):
            wo = []
            for c in range(2):
                w = woT.tile([128, IP], f16, tag=f"wo{c}", name=f"wo{c}")
                nc.sync.dma_start_transpose(
                    out=w[:], in_=wout16[:, c * 128 : (c + 1) * 128]
                )
                wo.append(w)
            nblocks = [(i, min(512, I - i)) for i in range(0, I, 512)]
            for t0 in range(0, T, 128):
                tr = min(128, T - t0)
                for n0, nw in nblocks:
                    ps = psO.tile([128, 512], f32, tag="ps_o", name="ps_o")
                    nc.tensor.matmul(
                        ps[:tr, :nw],
                        ones_sb[:, :tr],
                        bout16[:, n0 : n0 + nw],
                        start=True,
                        stop=False,
                    )
                    for c in range(2):
                        nc.tensor.matmul(
                            ps[:tr, :nw],
                            hs_v[:, t0 : t0 + tr, c],
                            wo[c][:, n0 : n0 + nw],
                            start=False,
                            stop=(c == 1),
                        )
                    res = ot.tile([128, 512], f32, tag="res", name="res")
                    nc.scalar.copy(res[:tr, :nw], ps[:tr, :nw])
                    nc.sync.dma_start(
                        out_v[t0 : t0 + tr, n0 : n0 + nw], res[:tr, :nw]
                    )
    _split_multi_waits(nc, mybir)
    return nc


def _get_nc(T=T, U=16):
    key = (T, U)
    if key not in _CACHE:
        import concourse.bass as bass
        import concourse.mybir as mybir
        import concourse.tile as tile

        nc = bass.Bass(trn_type="TRN2", debug=False)
        _CACHE[key] = _build(nc, bass, mybir, tile, T, U)
    return _CACHE[key]


def kernel(**inputs):
    nc = _get_nc()
    names = [
        "input_trajectory",
        "W_ih_enc", "W_hh_enc", "b_ih_enc", "b_hh_enc",
        "W_ih_dec", "W_hh_dec", "b_ih_dec", "b_hh_dec",
        "W_out", "b_out",
    ]
    in_map = {k: np.ascontiguousarray(np.asarray(inputs[k], dtype=np.float32)) for k in names}
    from concourse.bass_utils import run_bass_kernel_spmd

    res = run_bass_kernel_spmd(nc, [in_map], core_ids=[0])
    return res.results[0]["output"]
